# revision 45
# baseline (speedup 1.0000x reference)
"""Trainium2 Bass kernel for nn_Attention_39934605918652.

res[b] = W0 @ x0[b] + sum_{n=1..N-1} W2 @ tanh(W1a @ x0[b] + W1b @ x[b,n])

Algebraic optimization: W2 is n-independent, so
    sum_n W2 @ tanh(...) = W2 @ (sum_n tanh(...))
leaving one [B,H]x[H,F] epilogue matmul.

Sharding: data-parallel over batch B=128 across 8 cores (16 batches/core),
weights replicated. No collectives.

Design (vs the 88us bf16 baseline; measured ~78-80us, device-thermal
dependent). The baseline's three dominant costs were the bf16 matmul
stream (~55us of PE), 129 per-batch ACT tanh calls (~60us), and the DVE
segmented reduce (~39us). This version:
 - Runs the main matmuls in fp8e4m3 with MatmulPerfMode.DoubleRow (2
   rows/cycle): operands packed [128, ksub, cols] with contraction
   f = ksub*128 + p; each instruction contracts a ksub-pair (256
   features) over 512 psum columns (a matmul output may not cross a
   2KB PSUM bank). W1b is pre-scaled by 32 on host so its ~N(0, 1/1024)
   entries sit in e4m3's normal range; the tanh ACT applies scale=1/32.
 - Injects the h0 bias INTO PSUM with one selector matmul per bank:
   lhsT = h0T[16 batches, 128 h] (phase 1's natural psum layout, legal
   partition start), rhs = a host 0/1 fp16 matrix routing batch
   4q+2bk -> cols [0:255) and 4q+2bk+1 -> [256:511). Pad columns stay
   exactly 0 (tanh(0)=0), so the reduce needs no masking, and ACT needs
   no per-batch bias calls. start=True on the selector matmul doubles as
   the bank zeroing.
 - tanh is ONE big ACT call per (h,q) tile [128,1024], PSUM -> bf16.
 - The reduce is a Pool/DVE-alternating halving add (cols [128:256) onto
   [0:128)) + a half-size DVE segmented reduce_sum (TensorReduce has no
   2x mode; Pool measures ~2.25ns/elem on TENSOR_TENSOR vs DVE 2x_1p
   ~0.5ns/elem, hence the split).
 - W2 epilogue matmuls issue mid-stream as each S[h] completes (h-pair-
   major tile order); W0's terms share the same psum accumulation group.
 - Tiles run in (same-h, adjacent-q) pairs with both tiles' K=16
   selector matmuls batched ahead of the K=128 DR matmuls (fewer PE
   contraction-width transitions).
PE-cycle accounting (per core, full 2.4GHz clock): DR 65536 + bias
32768 (1 cycle/psum-column is the floor on ANY engine) + phase-1 8192 +
epilogue 6144 = ~47us; the rest is preamble (~6us), the DMA-shadowed
lead-in, and the tail chain. Device-thermal throttling adds up to ~20%.
Hardware scheduling lessons baked in:
 - HAM grants the PE full clock only after ~6us of sustained activity
   and may revoke it for tens of us after a multi-us idle gap, so dummy
   fp32 matmuls bridge every lead-in gap (N_WARM before phase 1 while
   weights stream, N_WARM2 between phase 1 and the prebias selectors
   while the h0T ACT copy completes).
 - All descriptor sets on a DMA queue share its 16 engines concurrently,
   so every tensor completes near the end of the whole stream; tiny gate
   reads stall the sync queue so first-needed tensors (wa -> phase 1,
   xi0/xi1 -> first tiles) finish early.
 - x0+W1a ride one packed dram tensor (wa) and W2+W0 another (wb): one
   descriptor per partition and 8-12KB contiguous runs.
Measured end-to-end rel err vs a float64 oracle: ~2.5e-3 (fp8 noise
averages down over the 255-term n-sum; harness gate is 2e-2).

Device layout (per core):
  xiT  [128, 4q*(4k*1024c)] fp8  col c = 256*b_in_q + n, f = k*128 + p
  w1bT [128, 4k*1024h]      fp8  = 32*W1b.T packed like xiT
  waT  [128, 64 + 4k*1024]  fp16 cols 0:64 x0 packed [p, f*16+b];
                                 rest W1a.T packed [p, k*1024+h]
  wbT  [128, 12*512]        fp16 8 W2.T h-tiles then 4 W0.T f-tiles
  selT [16, 8*512]          fp16 0/1 bias routing per (q, bank)
Output res [BL=16, 512] f32 per core; host concatenates.
"""

import os
import numpy as np
from contextlib import ExitStack

import concourse.bass as bass
import concourse.tile as tile
from concourse import bacc, mybir
from concourse.bass_utils import run_bass_kernel_spmd

N_CORES = 8
B, N, F, H = 128, 256, 512, 1024
BL = B // N_CORES          # 16 batches per core
NI = N - 1                 # 255 real columns per batch
NP = 256                   # padded columns per batch
NF = F // 128              # 4 f-chunks (= ksubs)
NH = H // 128              # 8 h-tiles
QUADS = BL // 4            # 4 batch-quads; per quad psum tile [128, 4*256]
W1B_SCALE = 32.0           # host pre-scale on W1b before fp8 quantization

F32 = mybir.dt.float32
BF16 = mybir.dt.bfloat16
F16 = mybir.dt.float16
F8 = mybir.dt.float8e4
DR = mybir.MatmulPerfMode.DoubleRow

# How many of the first psum tiles get their bias matmuls issued up front:
# they depend only on h0 (ready ~3us in), so they keep the PE busy/warm
# while xi still streams from HBM. Max useful = psum bufs.
N_PREBIAS = int(os.environ.get("KB_PREBIAS", "3"))
# Dummy fp32 matmuls on zeros (no DMA dependency): HAM only grants the PE
# full clock after ~6us of sustained activity and re-throttles to half
# rate after a multi-us idle gap, so the lead-in gaps are bridged with
# busywork. N_WARM runs before phase 1 (PE idle 6->11us while w1a
# streams); N_WARM2 runs after the prebias block (PE idle ~14->17us
# while xi0 streams). Each is ~427ns warm / ~850ns at half rate.
N_WARM = int(os.environ.get("KB_WARM", "15"))
N_WARM2 = int(os.environ.get("KB_WARM2", "4"))


def _build_kernel():
    nc = bacc.Bacc(
        "TRN2", target_bir_lowering=False, debug=False, num_devices=N_CORES
    )

    xiT = nc.dram_tensor("xiT", [128, QUADS * NF * 1024], F8, kind="ExternalInput").ap()
    w1bT = nc.dram_tensor("w1bT", [128, NF * H], F8, kind="ExternalInput").ap()
    waT = nc.dram_tensor("waT", [128, 64 + NF * 1024], F16, kind="ExternalInput").ap()
    wbT = nc.dram_tensor("wbT", [128, 12 * 512], F16, kind="ExternalInput").ap()
    selT = nc.dram_tensor("selT", [BL, 8 * 512], F16, kind="ExternalInput").ap()
    res = nc.dram_tensor("res", [BL, F], F32, kind="ExternalOutput").ap()

    with tile.TileContext(nc) as tc:
        with ExitStack() as ctx:
            _kernel_body(ctx, tc, xiT, w1bT, waT, wbT, selT, res)

    nc.compile()
    return nc


def _kernel_body(ctx, tc, xiT, w1bT, waT, wbT, selT, res):
    nc = tc.nc
    Tanh = mybir.ActivationFunctionType.Tanh
    Copy = mybir.ActivationFunctionType.Copy

    wpool = ctx.enter_context(tc.tile_pool(name="weights", bufs=1))

    # DMA issue order = first-need order. x0+w1a ride one packed tensor
    # (wa) and w2+w0 another (wb): one descriptor-gen each and 8-12KB
    # per-partition runs instead of many small strided transfers.
    wa = wpool.tile([128, 64 + NF * 1024], F16, tag="wa", name="wa")
    nc.sync.dma_start(wa[:], waT[:])
    sel_sb = wpool.tile([BL, 8 * 512], F16, tag="sel", name="sel")
    nc.sync.dma_start(sel_sb[:], selT[:])
    x0_sb = [wa[:, f * BL : (f + 1) * BL] for f in range(NF)]  # cols 0:64
    w1a_sl = lambda f, hh: wa[:, 64 + f * 1024 + hh * 512 : 64 + f * 1024 + (hh + 1) * 512]
    # Tiny gate reads stall the sync queue until an earlier tensor
    # completes, so later descriptor sets don't steal bandwidth from
    # first-needed tensors.
    gate16 = wpool.tile([1, 64], F16, tag="gate16", name="gate16")
    gate8 = wpool.tile([1, 64], F8, tag="gate8", name="gate8")
    _gate_n = [0]

    def gate_on(tile_ap):
        g = _gate_n[0]
        _gate_n[0] += 1
        dst = gate8 if tile_ap.dtype == F8 else gate16
        nc.sync.dma_start(dst[0:1, g * 4 : g * 4 + 2], tile_ap)
    w1b_all = wpool.tile([128, NF * H], F8, tag="w1b", name="w1b_all")
    nc.sync.dma_start(w1b_all[:], w1bT[:])
    w1b_v = w1b_all[:].rearrange("p (k h) -> p k h", k=NF)
    # xi0 joins the first transfer window (it binds the first DR matmul,
    # arriving later than the whole phase-1 chain when gated); xi1+ stay
    # behind the wa gate so they don't steal the window's bandwidth.
    xi_sb = []
    xi_t = []
    for q in range(QUADS):
        t = wpool.tile([128, NF * 1024], F8, tag=f"xi_{q}", name=f"xi_{q}")
        xi_t.append(t)
        base = q * NF * 1024
        nc.sync.dma_start(t[:], xiT[:, base : base + NF * 1024])
        xi_sb.append(t[:].rearrange("p (k c) -> p k c", k=NF))
        if q == 0:
            gate_on(wa[0:1, 0:2])
        if q == 1:
            gate_on(xi_t[0][0:1, 0:2])
    gate_on(xi_t[2][0:1, 0:2])
    wb = wpool.tile([128, 12 * 512], F16, tag="wb", name="wb")
    nc.sync.dma_start(wb[:], wbT[:])
    w2_sb = [wb[:, h * 512 : (h + 1) * 512] for h in range(NH)]
    w0_sb = [wb[:, (NH + f) * 512 : (NH + f + 1) * 512] for f in range(NF)]

    S_sb = [
        wpool.tile([128, BL], F16, tag=f"S_{h}", name=f"S_{h}")
        for h in range(NH)
    ]
    h0T_sb = wpool.tile([BL, H], F16, tag="h0T", name="h0T")

    # PSUM: main pool 3 x [128,1024]f32 (2 banks each) for the wave tiles
    # and ph0; small pool 2 x 1 bank for warm-up + the epilogue
    # accumulator. 3*2 + 2 = 8 banks.
    ppool = ctx.enter_context(tc.tile_pool(name="ps", bufs=3, space="PSUM"))
    spool = ctx.enter_context(tc.tile_pool(name="pss", bufs=2, space="PSUM"))
    itpool = ctx.enter_context(tc.tile_pool(name="it", bufs=6))

    # ---- Phase 0: preload the tanh ACT table during the DMA lead-in
    # (first ACTIVATE otherwise pays the ~1.3us table load mid-kernel).
    tiny = wpool.tile([128, 1], F32, tag="tiny", name="tiny")
    nc.gpsimd.memset(tiny[:], 0.0)
    nc.scalar.activation(tiny[:], tiny[:], Tanh)

    # ---- Phase 0b: PE warm-up while wa streams (see N_WARM).
    wz = wpool.tile([128, 256], F32, tag="warmz", name="warmz")
    nc.gpsimd.memset(wz[:], 0.0)
    pw = spool.tile([128, 256], F32, tag="pss", name="pwarm")
    for _ in range(N_WARM):
        nc.tensor.matmul(pw[:], wz[:, :128], wz[:], start=True, stop=True)

    # ---- Phase 1 (batch-major): h0T[b,h] = sum_f x0T[f,b] W1aT[f,h];
    # one ACT copy casts to fp16 with the 32x psum scale baked in. The
    # [16, 1024] batch-on-partitions layout is consumed directly as the
    # bias selector matmuls' lhsT (legal partition start 0).
    ph0 = ppool.tile([BL, H], F32, tag="ps", name="ph0")
    for hh in range(2):
        sl = slice(hh * 512, (hh + 1) * 512)
        for f in range(NF):
            nc.tensor.matmul(
                ph0[:, sl],
                x0_sb[f],
                w1a_sl(f, hh),
                start=(f == 0),
                stop=(f == NF - 1),
            )
    nc.scalar.activation(h0T_sb[:], ph0[:], Copy, scale=W1B_SCALE)


    def bias_mms(pb, h, q):
        # One selector matmul per PSUM bank: lhsT = h0T[16 batches, 128 h]
        # (phase-1's natural layout), rhs = a host 0/1 matrix routing batch
        # 4q+2bk to cols [0:255) and 4q+2bk+1 to [256:511) of the bank
        # (pad cols all-zero, so tanh(0)=0 and the reduce needs no mask).
        # Writes the full bank with start=True, zeroing it for the DR
        # accumulation. 2 instructions/tile, no flatten DMA, K=16.
        for bk in range(2):
            m = 2 * q + bk
            nc.tensor.matmul(
                pb[:, bk * 512 : (bk + 1) * 512],
                h0T_sb[:, h * 128 : (h + 1) * 128],
                sel_sb[:, m * 512 : (m + 1) * 512],
                start=True,
                stop=False,
                skip_group_check=True,
            )

    def main_mms(pb, h, q):
        # fp8 DoubleRow: 2 ksub-pairs x 2 col-halves (a matmul output may
        # not cross a PSUM bank), each contracting 256 features over 512
        # psum columns. kp outer so consecutive matmuls share lhsT.
        for kp in range(2):
            for bk in range(2):
                cols = slice(bk * 512, (bk + 1) * 512)
                nc.tensor.matmul(
                    pb[:, cols],
                    w1b_v[:, 2 * kp : 2 * kp + 2, h * 128 : (h + 1) * 128],
                    xi_sb[q][:, 2 * kp : 2 * kp + 2, cols],
                    start=False,
                    stop=(kp == 1),
                    perf_mode=DR,
                    skip_group_check=True,
                )

    def consume(h, q, pb, idx):
        # ACT: one big tanh over the whole psum tile (pad cols are exact
        # zeros). A halving add folds cols [128:256) onto [0:128), then a
        # half-size DVE segmented reduce. Pool (gpsimd) measures
        # ~2.25ns/elem on TENSOR_TENSOR vs DVE 2x_1p at ~0.5ns/elem, so
        # adds alternate between them; free-axis reduce is DVE-only.
        it = itpool.tile([128, 4 * NP], BF16, tag="it", name=f"it_{h}_{q}")
        nc.scalar.activation(it[:], pb[:], Tanh, scale=1.0 / W1B_SCALE)
        v = it[:].rearrange("p (b n) -> p b n", b=4)
        half = itpool.tile([128, 4 * 128], BF16, tag="half", name=f"hf_{h}_{q}")
        hv = half[:].rearrange("p (b n) -> p b n", b=4)
        add_eng = nc.gpsimd if (idx % 2 == 0 and idx < 28) else nc.vector
        with nc.allow_low_precision(
            reason="S accumulated in 16-bit to feed the fp16 epilogue matmul"
        ):
            add_eng.tensor_add(hv, v[:, :, 0:128], v[:, :, 128:256])
            nc.vector.reduce_sum(
                S_sb[h][:, q * 4 : (q + 1) * 4],
                hv,
                axis=mybir.AxisListType.X,
            )

    # ---- Phase 2. Tile order: h-pair-major, wave-inner, so each h's four
    # quads finish early and its W2 epilogue matmul can issue mid-stream.
    # The first N_PREBIAS tiles' bias matmuls go up front (they only need
    # h0), and N_WARM2 dummies bridge the PE gap until xi0 lands.
    order = []
    for hp in range(0, NH, 2):
        for w in range(2):
            for h in (hp, hp + 1):
                for q in (2 * w, 2 * w + 1):
                    order.append((h, q))
    done_count = {h: 0 for h in range(NH)}
    w2_pending = []
    po_issued = [0]
    po = spool.tile([BL, F], F32, tag="pss", name="po")

    def po_mm(lhsT, rhs):
        # One shared 12-matmul accumulation group: 8 W2 terms issued as
        # each S[h] completes mid-stream, 4 W0 terms slotted in at idx 17.
        nc.tensor.matmul(
            po[:], lhsT, rhs,
            start=(po_issued[0] == 0),
            stop=(po_issued[0] == NH + NF - 1),
            skip_group_check=True,
        )
        po_issued[0] += 1

    def flush_w2():
        h = w2_pending.pop(0)
        po_mm(S_sb[h][:], w2_sb[h])

    # Warm-up bridge BEFORE the prebias selector matmuls: they wait on the
    # h0T copy (~1.2us of ACT latency after phase 1), and a PE idle gap
    # there can make HAM revoke the full-clock grant for tens of us.
    for _ in range(N_WARM2):
        nc.tensor.matmul(pw[:], wz[:, :128], wz[:], start=True, stop=True)
    pbs = {}
    for h, q in order[:N_PREBIAS]:
        pb = pbs[(h, q)] = ppool.tile([128, 4 * NP], F32, tag="ps", name=f"pb_{h}_{q}")
        bias_mms(pb, h, q)

    # Tiles are processed in (same-h, q-adjacent) pairs with both tiles'
    # K=16 selector matmuls batched before the K=128 DR matmuls: the PE
    # pays the array-reconfig penalty between contraction widths once per
    # pair instead of once per tile.
    for pidx in range(len(order) // 2):
        ia, ib = 2 * pidx, 2 * pidx + 1
        ha, qa = order[ia]
        hb, qb = order[ib]
        if w2_pending and ia >= 2:
            flush_w2()
        if (ha, qa) in pbs:
            pba = pbs.pop((ha, qa))
        else:
            pba = ppool.tile([128, 4 * NP], F32, tag="ps", name=f"pb_{ha}_{qa}")
            bias_mms(pba, ha, qa)
        if (hb, qb) in pbs:
            pbb = pbs.pop((hb, qb))
        else:
            pbb = ppool.tile([128, 4 * NP], F32, tag="ps", name=f"pb_{hb}_{qb}")
            bias_mms(pbb, hb, qb)
        main_mms(pba, ha, qa)
        consume(ha, qa, pba, ia)
        main_mms(pbb, hb, qb)
        consume(hb, qb, pbb, ib)
        for h in (ha, hb):
            done_count[h] += 1
            if done_count[h] == 4:
                w2_pending.append(h)
    # W0 epilogue terms are S-independent: issuing them HERE gives the PE
    # real work during the last tile's ACT->add->reduce drain, which
    # otherwise shows as a ~2us PE gap before the final W2 matmul.
    for f in range(NF):
        po_mm(x0_sb[f], w0_sb[f])
    while w2_pending:
        flush_w2()

    # ---- Phase 3 tail: one copy from PSUM (DMA cannot read PSUM), out.
    # ACT does this [16,512] copy in ~570ns vs DVE's ~813ns, and both are
    # idle here, so it shortens the critical tail chain.
    rt = itpool.tile([BL, F], F32, tag="rt", name="rt")
    nc.scalar.activation(rt[:], po[:], Copy)
    nc.sync.dma_start(res[:], rt[:])


_NC_CACHE = {}


def _get_nc():
    key = ("v21", N_PREBIAS, N_WARM, N_WARM2)
    if key not in _NC_CACHE:
        _NC_CACHE[key] = _build_kernel()
    return _NC_CACHE[key]


def _make_in_maps(x, W1, W2, W0):
    import ml_dtypes
    f8 = ml_dtypes.float8_e4m3
    f16 = np.float16

    x = np.ascontiguousarray(np.asarray(x, dtype=np.float32))
    W1 = np.asarray(W1, dtype=np.float32)
    W2 = np.asarray(W2, dtype=np.float32)
    W0 = np.asarray(W0, dtype=np.float32)

    # [p, k, h] = 32 * W1b[h, k*128+p]
    w1bT = np.ascontiguousarray(
        (W1[:, F:].T * W1B_SCALE).reshape(NF, 128, H).transpose(1, 0, 2).reshape(128, NF * H)
    ).astype(f8)
    # wb [p, j*512+g]: j<8 -> W2[g, j*128+p]; j>=8 -> W0[g, (j-8)*128+p]
    wb = np.empty((128, 12 * 512), dtype=np.float32)
    wb[:, : NH * 512] = W2.T.reshape(NH, 128, F).transpose(1, 0, 2).reshape(128, NH * F)
    wb[:, NH * 512 :] = W0.T.reshape(NF, 128, F).transpose(1, 0, 2).reshape(128, NF * F)
    wbT = np.ascontiguousarray(wb).astype(f16)
    sel = np.zeros((BL, 8 * 512), dtype=np.float32)
    for m in range(8):
        q, bk = divmod(m, 2)
        b0 = 4 * q + 2 * bk
        sel[b0, m * 512 : m * 512 + NI] = 1.0
        sel[b0 + 1, m * 512 + NP : m * 512 + NP + NI] = 1.0
    selT = np.ascontiguousarray(sel).astype(f16)

    in_maps = []
    for i in range(N_CORES):
        xc = x[i * BL : (i + 1) * BL]               # [BL, N, F]
        # wa cols 0:64 = x0 packed [p, f*BL+b]; cols 64: = W1a.T packed
        # [p, k*1024+h]
        wa = np.empty((128, 64 + NF * 1024), dtype=np.float32)
        wa[:, :64] = (
            xc[:, 0, :].T.reshape(NF, 128, BL).transpose(1, 0, 2).reshape(128, NF * BL)
        )
        wa[:, 64:] = (
            W1[:, :F].T.reshape(NF, 128, H).transpose(1, 0, 2).reshape(128, NF * H)
        )
        waT = np.ascontiguousarray(wa).astype(f16)
        pad = np.zeros((BL, NP, F), dtype=np.float32)
        pad[:, :NI, :] = xc[:, 1:, :]
        xiT_full = pad.reshape(BL * NP, F).T        # [512, 4096], col = 256*b + n
        # [p, q, k, c] = xiT_full[k*128+p, q*1024+c]
        xiTc = np.ascontiguousarray(
            xiT_full.reshape(NF, 128, QUADS, 1024)
            .transpose(1, 2, 0, 3)
            .reshape(128, QUADS * NF * 1024)
        ).astype(f8)
        in_maps.append(
            {
                "xiT": xiTc,
                "w1bT": w1bT,
                "waT": waT,
                "wbT": wbT,
                "selT": selT,
            }
        )
    return in_maps


def _gather(results):
    out = np.empty((B, F), dtype=np.float32)
    for i in range(N_CORES):
        out[i * BL : (i + 1) * BL] = results[i]["res"]
    return out


def kernel(x, W1, W2, W0):
    nc = _get_nc()
    in_maps = _make_in_maps(x, W1, W2, W0)
    res = run_bass_kernel_spmd(nc, in_maps, list(range(N_CORES)))
    return _gather(res.results)


def kernel_profiled(x, W1, W2, W0, **trace_kwargs):
    """Like kernel() but with NTFF profiling; returns (out, exec_time_ns)."""
    nc = _get_nc()
    in_maps = _make_in_maps(x, W1, W2, W0)
    res = run_bass_kernel_spmd(
        nc, in_maps, list(range(N_CORES)), trace=True, **trace_kwargs
    )
    return _gather(res.results), res.exec_time_ns
